# revision 4
# baseline (speedup 1.0000x reference)
"""Trainium2 Bass kernel for nn_AttentionLayer_13134009991917 (linear attention).

Reference math (per batch element):
    q = tanh(Wq @ query + bq)        [D=128, Tq=4096]
    k = tanh(Wk @ key  + bk)         [D=128, Tk=4096]
    v = tanh(Wv @ value + bv)        [M=128, Tk=4096]
    attn = q^T k                     [Tq, Tk]      (NO softmax)
    av[m,tq] = sum_tk attn[tq,tk] v[m,tk]
    out = tanh(Wa @ av + ba)         [M, Tq]

Since there is no softmax, associativity collapses the [Tq,Tk] matrix:
    KV = v @ k^T                     [M, D]   (contract Tk)
    av = KV @ q                      [M, Tq]
and Wa folds into KV (linear before the final tanh):
    W2 = Wa @ KV ;  out = tanh(W2 @ q + ba)

This drops the attention FLOPs ~32x and makes the problem memory-bound.

Sharding: B=8 batch elements -> one per NeuronCore, pure data parallel
(weights replicated). No collectives.

Per-core dataflow (all fp32):
    1. DMA weights; transpose Wq/Wk/Wv/Wa on PE (identity transpose).
    2. DMA key/value [128, 4096]; for each 512-wide tile:
       k_fm = tanh(WkT.T @ key_tile + bk) (feature-major, ACT fused bias),
       PE-transpose 128x128 chunks into tk-major k^T, v^T chunks,
       accumulate KV += v^T_chunk.T @ k^T_chunk in PSUM.
    3. W2T = KV.T-free form: W2T[d,m'] via matmul(lhsT=KV, rhs=WaT).
    4. For each 512-wide Tq tile: q = tanh(WqT.T @ query_tile + bq),
       z = W2T.T @ q, out_tile = tanh(z + ba), DMA out.
"""

import numpy as np

import concourse.bass as bass
import concourse.mybir as mybir
import concourse.tile as tile
from concourse import bacc
from concourse.bass import ts
from concourse.bass_utils import run_bass_kernel_spmd
from concourse.masks import make_identity

F32 = mybir.dt.float32
TANH = mybir.ActivationFunctionType.Tanh

B = 8
IN_SZ = 256      # query feature dim
D = 128          # q_sz (attention dim)
M = 128          # mem (value dim)
TQ = 4096
TK = 4096
P = 128          # partitions
TQT = 512        # Tq tile (fp32 moving-operand max)
NTQ = TQ // TQT  # 8
TKT = 512        # Tk tile for the feature-major dense
NTK = TK // TKT  # 8

# Matmul compute dtype: float32 (exact) or float32r (4x faster at >=256 free
# cols on a warm PE, precision to be validated on HW).
MM_DT = F32


def _mm(x):
    """View an f32 AP as the matmul compute dtype."""
    if MM_DT is F32:
        return x
    return x.bitcast(MM_DT)


def build_nc():
    # Bacc (not raw Bass): its compile() pass splits multi-sem waits into
    # EventSemaphore instructions — walrus allows only 1 sync wait per
    # Matmult/LDWEIGHTS ("Too many sync wait commands" otherwise).
    nc = bacc.Bacc()

    query = nc.declare_dram_parameter("query", [IN_SZ, TQ], F32, isOutput=False)
    key = nc.declare_dram_parameter("key", [M, TK], F32, isOutput=False)
    value = nc.declare_dram_parameter("value", [M, TK], F32, isOutput=False)
    Wq = nc.declare_dram_parameter("Wq", [D, IN_SZ], F32, isOutput=False)
    bq = nc.declare_dram_parameter("bq", [D, 1], F32, isOutput=False)
    Wk = nc.declare_dram_parameter("Wk", [D, M], F32, isOutput=False)
    bk = nc.declare_dram_parameter("bk", [D, 1], F32, isOutput=False)
    Wv = nc.declare_dram_parameter("Wv", [M, M], F32, isOutput=False)
    bv = nc.declare_dram_parameter("bv", [M, 1], F32, isOutput=False)
    Wa = nc.declare_dram_parameter("Wa", [M, M], F32, isOutput=False)
    ba = nc.declare_dram_parameter("ba", [M, 1], F32, isOutput=False)
    out = nc.declare_dram_parameter("out", [M, TQ], F32, isOutput=True)

    with tile.TileContext(nc) as tc:
        with (
            tc.tile_pool(name="consts", bufs=1) as consts,
            tc.tile_pool(name="bigio", bufs=1) as bigio,
            tc.tile_pool(name="qin", bufs=1) as qin_pool,
        ):
            # ---------------- constants + weight transposes ----------------
            ident = consts.tile([P, P], F32)
            make_identity(nc, ident)

            wq_sb = consts.tile([D, IN_SZ], F32)
            nc.sync.dma_start(wq_sb, Wq[:, :])
            wk_sb = consts.tile([D, M], F32)
            nc.sync.dma_start(wk_sb, Wk[:, :])
            wv_sb = consts.tile([M, M], F32)
            nc.sync.dma_start(wv_sb, Wv[:, :])
            wa_sb = consts.tile([M, M], F32)
            nc.sync.dma_start(wa_sb, Wa[:, :])

            bq_sb = consts.tile([D, 1], F32)
            nc.sync.dma_start(bq_sb, bq[:, :])
            bk_sb = consts.tile([D, 1], F32)
            nc.sync.dma_start(bk_sb, bk[:, :])
            bv_sb = consts.tile([M, 1], F32)
            nc.sync.dma_start(bv_sb, bv[:, :])
            ba_sb = consts.tile([M, 1], F32)
            nc.sync.dma_start(ba_sb, ba[:, :])

            # ---------------- big inputs ----------------
            key_sb = bigio.tile([M, TK], F32)
            value_sb = bigio.tile([M, TK], F32)
            for i in range(4):
                nc.sync.dma_start(key_sb[:, ts(i, TK // 4)], key[:, ts(i, TK // 4)])
                nc.sync.dma_start(
                    value_sb[:, ts(i, TK // 4)], value[:, ts(i, TK // 4)]
                )
            # query tiles: [f-chunk, tile] -> [128, 512] each, all resident
            qin = []
            for t in range(NTQ):
                q0 = qin_pool.tile([P, TQT], F32, name=f"qin0_{t}")
                nc.sync.dma_start(q0, query[0:P, ts(t, TQT)])
                q1 = qin_pool.tile([P, TQT], F32, name=f"qin1_{t}")
                nc.sync.dma_start(q1, query[P : 2 * P, ts(t, TQT)])
                qin.append((q0, q1))

            # transposed weights (PE identity transpose, psum -> sbuf copy)
            wqT0 = consts.tile([P, D], F32)
            wqT1 = consts.tile([P, D], F32)
            wkT = consts.tile([M, D], F32)
            wvT = consts.tile([M, M], F32)
            waT = consts.tile([M, M], F32)
            kv_sb = consts.tile([M, D], F32)
            w2T_sb = consts.tile([D, M], F32)

            with tc.tile_pool(name="ps_w", bufs=2, space="PSUM") as ps_w:
                for dst, src in (
                    (wqT0, wq_sb[:, 0:P]),
                    (wqT1, wq_sb[:, P : 2 * P]),
                    (wkT, wk_sb[:, :]),
                    (wvT, wv_sb[:, :]),
                    (waT, wa_sb[:, :]),
                ):
                    pt = ps_w.tile([P, P], F32, tag="wtr")
                    nc.tensor.transpose(pt, src, ident)
                    nc.vector.tensor_copy(dst, pt)

            # ---------------- k/v dense + transpose + KV accumulation ------
            with (
                tc.tile_pool(name="fm_sb", bufs=4) as fm_pool,
                tc.tile_pool(name="tchunk", bufs=8) as tchunk_pool,
                tc.tile_pool(name="ps_fm", bufs=2, space="PSUM") as ps_fm,
                tc.tile_pool(name="ps_tr", bufs=4, space="PSUM") as ps_tr,
                tc.tile_pool(name="ps_kv", bufs=1, space="PSUM") as ps_kv,
            ):
                kv_ps = ps_kv.tile([M, D], F32)
                n_acc = 0
                for t in range(NTK):
                    # k tile: [D, 512] = WkT.T @ key_tile
                    kfm_ps = ps_fm.tile([D, TKT], F32, tag="fm")
                    nc.tensor.matmul(
                        kfm_ps,
                        _mm(wkT[:, :]),
                        _mm(key_sb[:, ts(t, TKT)]),
                        start=True,
                        stop=True,
                    )
                    kfm = fm_pool.tile([D, TKT], F32, tag="kfm")
                    nc.scalar.activation(kfm, kfm_ps, TANH, bias=bk_sb[:, :])

                    vfm_ps = ps_fm.tile([M, TKT], F32, tag="fm")
                    nc.tensor.matmul(
                        vfm_ps,
                        _mm(wvT[:, :]),
                        _mm(value_sb[:, ts(t, TKT)]),
                        start=True,
                        stop=True,
                    )
                    vfm = fm_pool.tile([M, TKT], F32, tag="vfm")
                    nc.scalar.activation(vfm, vfm_ps, TANH, bias=bv_sb[:, :])

                    # transpose 128x128 chunks to tk-major and accumulate KV
                    for j in range(TKT // P):
                        ktp = ps_tr.tile([P, D], F32, tag="tr")
                        nc.tensor.transpose(ktp, kfm[:, ts(j, P)], ident)
                        ktc = tchunk_pool.tile([P, D], F32, tag="ktc")
                        nc.vector.tensor_copy(ktc, ktp)

                        vtp = ps_tr.tile([P, M], F32, tag="tr")
                        nc.tensor.transpose(vtp, vfm[:, ts(j, P)], ident)
                        vtc = tchunk_pool.tile([P, M], F32, tag="vtc")
                        nc.vector.tensor_copy(vtc, vtp)

                        n_acc += 1
                        nc.tensor.matmul(
                            kv_ps,
                            _mm(vtc[:, :]),
                            _mm(ktc[:, :]),
                            start=(n_acc == 1),
                            stop=(n_acc == NTK * (TKT // P)),
                            skip_group_check=True,
                        )

                nc.vector.tensor_copy(kv_sb, kv_ps)
                # W2T[d, m'] = sum_m KV[m, d] * Wa[m', m]
                w2_ps = ps_tr.tile([D, M], F32, tag="tr")
                nc.tensor.matmul(
                    w2_ps, _mm(kv_sb[:, :]), _mm(waT[:, :]), start=True, stop=True
                )
                nc.vector.tensor_copy(w2T_sb, w2_ps)

            # ---------------- q dense + output ----------------
            with (
                tc.tile_pool(name="qsb", bufs=3) as qsb_pool,
                tc.tile_pool(name="osb", bufs=3) as osb_pool,
                tc.tile_pool(name="ps_q", bufs=2, space="PSUM") as ps_q,
                tc.tile_pool(name="ps_z", bufs=2, space="PSUM") as ps_z,
            ):
                for t in range(NTQ):
                    q0, q1 = qin[t]
                    q_ps = ps_q.tile([D, TQT], F32, tag="q")
                    nc.tensor.matmul(
                        q_ps, _mm(wqT0[:, :]), _mm(q0[:, :]), start=True, stop=False
                    )
                    nc.tensor.matmul(
                        q_ps, _mm(wqT1[:, :]), _mm(q1[:, :]), start=False, stop=True
                    )
                    q_sb = qsb_pool.tile([D, TQT], F32, tag="qsb")
                    nc.scalar.activation(q_sb, q_ps, TANH, bias=bq_sb[:, :])

                    z_ps = ps_z.tile([M, TQT], F32, tag="z")
                    nc.tensor.matmul(
                        z_ps, _mm(w2T_sb[:, :]), _mm(q_sb[:, :]), start=True, stop=True
                    )
                    o_sb = osb_pool.tile([M, TQT], F32, tag="osb")
                    nc.scalar.activation(o_sb, z_ps, TANH, bias=ba_sb[:, :])
                    nc.sync.dma_start(out[:, ts(t, TQT)], o_sb)

    nc.finalize()
    return nc


_CACHED_NC = None


def _get_nc():
    global _CACHED_NC
    if _CACHED_NC is None:
        _CACHED_NC = build_nc()
    return _CACHED_NC


def make_in_maps(inputs):
    in_maps = []
    for b in range(B):
        in_maps.append(
            {
                "query": np.ascontiguousarray(inputs["query"][b], dtype=np.float32),
                "key": np.ascontiguousarray(inputs["key"][b], dtype=np.float32),
                "value": np.ascontiguousarray(inputs["value"][b], dtype=np.float32),
                "Wq": np.ascontiguousarray(inputs["Wq"], dtype=np.float32),
                "bq": np.ascontiguousarray(
                    np.reshape(inputs["bq"], (D, 1)), dtype=np.float32
                ),
                "Wk": np.ascontiguousarray(inputs["Wk"], dtype=np.float32),
                "bk": np.ascontiguousarray(
                    np.reshape(inputs["bk"], (D, 1)), dtype=np.float32
                ),
                "Wv": np.ascontiguousarray(inputs["Wv"], dtype=np.float32),
                "bv": np.ascontiguousarray(
                    np.reshape(inputs["bv"], (M, 1)), dtype=np.float32
                ),
                "Wa": np.ascontiguousarray(inputs["Wa"], dtype=np.float32),
                "ba": np.ascontiguousarray(
                    np.reshape(inputs["ba"], (M, 1)), dtype=np.float32
                ),
            }
        )
    return in_maps


def run(inputs, trace=False, **kwargs):
    nc = _get_nc()
    res = run_bass_kernel_spmd(
        nc, make_in_maps(inputs), core_ids=list(range(B)), trace=trace, **kwargs
    )
    out = np.stack(
        [np.asarray(res.results[i]["out"], dtype=np.float32) for i in range(B)], axis=0
    )
    return out, res


def kernel(**inputs):
    out, _ = run(inputs, trace=False)
    return out
